# revision 2
# baseline (speedup 1.0000x reference)
"""Trainium2 Bass kernel for nn_DotProductAttention (B=8, LQ=LK=4096, F=64).

Reference computation:
    q = query @ wq.T + bq ; k = key @ wk.T + bk ; v = value @ wv.T + bv
    scores = einsum('bkf,bqf->bkq', k, q)
    attn = softmax(scores, axis=-1)           # over q positions
    out = einsum('bkq,bqf->bkf', attn, v)

Strategy: batch b -> core b (8 cores, no cross-core communication).

Algebraic folding (host side, O(L*F) prep only -- all O(L^2) work on device):
    scores[k,q] = (wk x_k + bk).(wq x_q + bq)
    The per-k term is constant along the softmax axis (q) and cancels, so with
    M = wq^T wk, c = wq^T bk the transposed scores are
        S^T[q,k] = query[q,:] @ ktil[:,k],   ktil = M @ key^T + c   (host)
    Softmax rows sum to 1, so the v-projection commutes with attention:
        out = (attn @ value) @ wv.T + bv
    U^T = [value | 1]^T @ exp(S^T) accumulates in PSUM; its last row is the
    softmax denominator l. The tiny projection (U/l) @ wv.T + bv runs on host.

Device loop (per core): for each 512-wide k-chunk, sweep the 32 q-blocks.
The exp() of the 16.7M scores is the kernel bottleneck (ACT engine: 1
elem/cycle/lane @1.2GHz = 109us alone), so exp is SPLIT between two engines:
  - ACT path (24 of 32 slots/chunk): fp32 PSUM supertiles of 3x512 slots,
    one exact Exp ACTIVATE each -> bf16 SBUF.
  - DVE path (8 of 32 slots/chunk, period 4): one tensor_scalar per slot
    computes i16 = round(s * 128*log2(e) + B) from PSUM fp32; the int16
    buffer IS the bf16 bit pattern of ~2^(s*log2 e) = e^s (Schraudolph:
    bit-level exp2 with linear mantissa interpolation, ~3.4% max elem err,
    ~1e-2 output err contribution at 25% routing -- fits rel-err budget).
P@V is one K=128 matmul per (chunk, j) with vaug stationary (LDW hides in
the background weight slot); scores matmuls alternate 64-row PE groups so
adjacent matmuls row-tile concurrently. PSUM budget: 2x3 banks score
supertiles + 1 bank DVE slot + 1 bank P@V accumulator = 8.
"""

import numpy as np
import ml_dtypes

import concourse.mybir as mybir
import concourse.tile as tile
from concourse import bacc
from concourse.bass_utils import run_bass_kernel_spmd
from concourse.vector_clock import ScopedClock


class _FastExitTileContext(tile.TileContext):
    """TileContext whose exit skips the second all-engine barrier.

    The final barrier only orders the gpsimd semaphore-clears against the
    other engines' completion; NEFF execution completion already waits for
    every engine's last instruction, and the clears still run, so repeated
    executions stay correct. Saves ~2-3us of kernel tail.
    """

    def _drain_and_barrier(self, tick_clock, wait_clock):
        drain_inst = self.nc.sync.drain()
        wait_clock.add_sem_waits(
            drain_inst.ins, ScopedClock({None: tick_clock.global_clock})
        )
        self.nc.all_engine_barrier()
        popped = self.nc._tile_sem_poison_stack.pop()
        assert popped is self._sem_poison
        self.nc.clear_and_free_semaphores(list(self.sems.allocated().values()))


F32 = mybir.dt.float32
F16 = mybir.dt.float16
BF16 = mybir.dt.bfloat16
I16 = mybir.dt.int16

L = 4096          # sequence length (both q and k)
F = 64            # feature dim
NBLK = L // 32    # unused marker (kept name parity)
NJ = 32           # q-blocks of 128
CHW = 512         # k-chunk width
NCH = 8           # number of k-chunks

# DVE Schraudolph constants: i16 = round(s * SCHRAU_SCALE + SCHRAU_B)
SCHRAU_SCALE = float(128.0 * np.log2(np.e))   # 184.664...
SCHRAU_B = float(16256 - 6)                   # 127<<7 minus sawtooth-centering

LAG = 8           # P@V emission lag in slots


def chunk_layout(c):
    """Per-chunk slot routing: list of (kind, slots) groups in slot order.

    kind 'A' -> ACT exact-exp supertile (1-3 slots), 'D' -> DVE Schraudolph
    single slot. Pattern: [A A A D] x 8 (chunk 0 leads with three 1-slot ACT
    groups to prime the pipeline quickly).
    """
    groups = []
    if c == 0:
        groups += [("A", [0]), ("A", [1]), ("A", [2]), ("D", [3])]
        s = 4
    else:
        s = 0
    while s < NJ:
        groups.append(("A", [s, s + 1, s + 2]))
        groups.append(("D", [s + 3]))
        s += 4
    return groups


def build_nc():
    nc = bacc.Bacc(None, target_bir_lowering=False)

    xqT = nc.dram_tensor("xqT", [128, L // 2], F16, kind="ExternalInput")
    ktil = nc.dram_tensor("ktil", [128, L], F16, kind="ExternalInput")
    vaug = nc.dram_tensor("vaug", [128, NJ * (F + 1)], BF16, kind="ExternalInput")
    uout = nc.dram_tensor("uout", [F + 1, L], F32, kind="ExternalOutput")

    Exp = mybir.ActivationFunctionType.Exp
    Mult = mybir.AluOpType.mult
    Add = mybir.AluOpType.add

    # slot -> (kind, group_key, offset, group_size, is_last)
    layouts = {}
    for c in range(NCH):
        smap = {}
        for gi, (kind, slots) in enumerate(chunk_layout(c)):
            for off, s in enumerate(slots):
                smap[s] = (kind, (c, gi), off, len(slots), off == len(slots) - 1)
        layouts[c] = smap

    with _FastExitTileContext(nc) as tc:
        with (
            tc.tile_pool(name="persist", bufs=1) as persist,
            tc.tile_pool(name="pt", bufs=6) as ptpool,
            tc.tile_pool(name="dpt", bufs=6) as dptpool,
            tc.tile_pool(name="utbf", bufs=2) as utbfpool,
            tc.tile_pool(name="ps_st", bufs=2, space="PSUM") as ps_st,
            tc.tile_pool(name="ps_dve", bufs=1, space="PSUM") as ps_dve,
            tc.tile_pool(name="ps_ut", bufs=1, space="PSUM") as ps_ut,
        ):
            # Warm the ACT exp table while input DMAs are still in flight:
            # a tiny exp on a memset tile forces the exp_and_others
            # ACT_TABLE_LOAD (~2.7us) off the critical path.
            warm_in = persist.tile([128, 8], F32)
            warm_out = persist.tile([128, 8], BF16)
            nc.vector.memset(warm_in[:], 0.0)
            nc.scalar.activation(warm_out[:], warm_in[:], Exp)

            # Split DMAs so the first iterations' inputs land early; the
            # j=0 row-half quarters go absolutely first.
            xqT_sb = persist.tile([128, L // 2], F16)
            ktil_sb = persist.tile([128, L], F16)
            vaug_sb = persist.tile([128, NJ * (F + 1)], BF16)
            nc.sync.dma_start(xqT_sb[0:64, 0:128], xqT[0:64, 0:128])
            nc.sync.dma_start(ktil_sb[0:64, 0:CHW], ktil[0:64, 0:CHW])
            nc.sync.dma_start(xqT_sb[64:128, 0:128], xqT[64:128, 0:128])
            nc.sync.dma_start(ktil_sb[64:128, 0:CHW], ktil[64:128, 0:CHW])
            nc.sync.dma_start(vaug_sb[:, 0:2 * (F + 1)], vaug[:, 0:2 * (F + 1)])
            nc.sync.dma_start(xqT_sb[:, 128:], xqT[:, 128:])
            nc.sync.dma_start(vaug_sb[:, 2 * (F + 1):], vaug[:, 2 * (F + 1):])
            nc.sync.dma_start(ktil_sb[:, CHW:], ktil[:, CHW:])

            sts = {}   # group_key -> psum tile being filled
            pts = {}   # group_key -> SBUF exp tile (bf16, or i16-as-bf16)
            uts = {}   # chunk -> psum accumulator

            def emit_scores(c, s):
                kind, key, off, gsz, last = layouts[c][s]
                if off == 0:
                    pool = ps_st if kind == "A" else ps_dve
                    sts[key] = pool.tile([128, 512 * gsz], F32,
                                         name="st", tag="st" + kind)
                st = sts[key]
                rh = 64 * (s % 2)
                qcols = slice(128 * (s // 2), 128 * (s // 2 + 1))
                kcols = slice(CHW * c, CHW * (c + 1))
                nc.tensor.matmul(st[:, 512 * off: 512 * (off + 1)],
                                 xqT_sb[rh:rh + 64, qcols],
                                 ktil_sb[rh:rh + 64, kcols],
                                 start=True, stop=True, tile_position=(rh, 0))
                if last:
                    stt = sts.pop(key)
                    if kind == "A":
                        pt = ptpool.tile([128, 512 * gsz], BF16,
                                         name="pt", tag="pt")
                        nc.scalar.activation(pt[:], stt[:], Exp)
                    else:
                        pt = dptpool.tile([128, 512], I16, name="dpt", tag="dpt")
                        nc.vector.tensor_scalar(
                            pt[:], stt[:], SCHRAU_SCALE, SCHRAU_B, Mult, Add)
                    pts[key] = pt

            def emit_pav(c, s):
                if s == 0:
                    uts[c] = ps_ut.tile([F + 1, CHW], F32, name="ut", tag="ut")
                ut = uts[c]
                kind, key, off, gsz, last = layouts[c][s]
                pt = pts[key]
                rhs = (pt[:, 512 * off: 512 * (off + 1)] if kind == "A"
                       else pt[:].bitcast(BF16))
                vsl = slice((F + 1) * s, (F + 1) * (s + 1))
                nc.tensor.matmul(ut[:], vaug_sb[:, vsl], rhs,
                                 start=(s == 0), stop=(s == NJ - 1))
                if last:
                    pts.pop(key)

            def emit_epilogue(c):
                ut = uts.pop(c)
                utbf = utbfpool.tile([F + 1, CHW], F32)
                nc.vector.tensor_copy(utbf[:], ut[:])
                nc.sync.dma_start(uout[:, CHW * c: CHW * (c + 1)], utbf[:])

            NTOT = NCH * NJ
            for gstep in range(NTOT + LAG):
                if gstep < NTOT:
                    emit_scores(gstep // NJ, gstep % NJ)
                if gstep >= LAG:
                    pc, ps = (gstep - LAG) // NJ, (gstep - LAG) % NJ
                    emit_pav(pc, ps)
                    if ps == NJ - 1:
                        emit_epilogue(pc)

    nc.compile()
    return nc


def host_pack(query_b, key_b, value_b, M, c):
    """Per-batch device-input packing (numpy, O(L*F))."""
    qT = query_b.T.reshape(F, L // 256, 2, 128)
    xqT = np.ascontiguousarray(                                       # [128, L/2]
        np.concatenate([qT[:, :, 0, :], qT[:, :, 1, :]], axis=0)
        .reshape(128, L // 2)).astype(np.float16)
    kt = (M @ key_b.T + c[:, None]).astype(np.float16)                # [64, L]
    ktil = np.ascontiguousarray(np.concatenate([kt, kt], axis=0))     # [128, L]
    v3 = value_b.reshape(NJ, 128, F).transpose(1, 0, 2)               # [128, NJ, F]
    vaug = np.ones((128, NJ, F + 1), np.float32)
    vaug[:, :, 0:F] = v3
    vaug_bf = vaug.reshape(128, NJ * (F + 1)).astype(ml_dtypes.bfloat16)
    return xqT, ktil, np.ascontiguousarray(vaug_bf)


def host_consts(wq, bq, wk, bk, wv, bv):
    wq64 = wq.astype(np.float64)
    M = (wq64.T @ wk.astype(np.float64)).astype(np.float32)
    c = (wq64.T @ bk.astype(np.float64)).astype(np.float32)
    return M, c


_NC = None


def kernel(**inputs):
    out, _ = run_kernel(inputs)
    return out


def run_kernel(inputs, **spmd_kwargs):
    global _NC
    if _NC is None:
        _NC = build_nc()

    query = np.asarray(inputs["query"], np.float32)
    key = np.asarray(inputs["key"], np.float32)
    value = np.asarray(inputs["value"], np.float32)
    wv = np.asarray(inputs["wv"], np.float32)
    bv = np.asarray(inputs["bv"], np.float32)
    M, c = host_consts(
        np.asarray(inputs["wq"], np.float32), np.asarray(inputs["bq"], np.float32),
        np.asarray(inputs["wk"], np.float32), np.asarray(inputs["bk"], np.float32),
        wv, bv)

    B = query.shape[0]
    in_maps = []
    for b in range(B):
        xqT, ktil, vaug = host_pack(query[b], key[b], value[b], M, c)
        in_maps.append({"xqT": xqT, "ktil": ktil, "vaug": vaug})
    res = run_bass_kernel_spmd(_NC, in_maps, core_ids=list(range(B)), **spmd_kwargs)
    outs = []
    for b in range(B):
        u = res.results[b]["uout"]              # [65, L] fp32: U^T rows + l row
        ut = (u[0:F, :] / u[F:F + 1, :]).T      # [L, F] normalized attention @ value
        outs.append(ut @ wv.T + bv)             # host fp32 epilogue projection
    out = np.stack(outs).astype(np.float32)
    return out, res


# revision 5
# speedup vs baseline: 1.0571x; 1.0571x over previous
"""Trainium2 Bass kernel for nn_DotProductAttention (B=8, LQ=LK=4096, F=64).

Reference computation:
    q = query @ wq.T + bq ; k = key @ wk.T + bk ; v = value @ wv.T + bv
    scores = einsum('bkf,bqf->bkq', k, q)
    attn = softmax(scores, axis=-1)           # over q positions
    out = einsum('bkq,bqf->bkf', attn, v)

Strategy: batch b -> core b (8 cores, no cross-core communication).

Algebraic folding (host side, O(L*F) prep only -- all O(L^2) work on device):
    with M = wq^T wk, c = wq^T bk (per-k softmax-invariant term dropped):
        S^T[q,k] = query[q,:] @ ktil[:,k],   ktil = M @ key^T + c   (host)
    U^T = [value | 1]^T @ exp(S^T) accumulates in PSUM; its last row is the
    softmax denominator l. The tiny projection (U/l) @ wv.T + bv runs on host.

Device loop (per core): 8 k-chunks of 512 columns; per chunk sweep the 32
q-blocks (slots). The exp() of the 16.7M scores is the kernel bottleneck
(ACT engine: 1 elem/cycle/lane @1.2GHz = 109us alone), so exp is SPLIT:
  - ACT path: exact Exp ACTIVATEs over the leading banks of each PSUM
    supertile -> bf16 SBUF.
  - DVE path (~10 of 32 slots/chunk): the third bank of an "M" supertile is
    processed by one tensor_scalar: i16 = round(s * 128*log2(e) + B) from
    PSUM fp32 [128,512]; the int16 buffer IS the bf16 bit pattern of ~e^s
    (Schraudolph bit-trick exp2, ~3.4% max elem err; ~1.2e-2 output err at
    ~30% routing -- inside the 2e-2 budget). ScalarE/VectorE read different
    PSUM banks of the same tile in parallel (legal on TRN2).
P@V lags LAG slots behind scores: two concurrent 64-row-group matmuls
(tile_position row tiling) accumulate q-halves into utl/uth (separate PSUM
banks -- concurrent drains into one bank are fatal). Host adds the halves.
PSUM: 2x3 supertile banks + utl + uth = 8.
"""

import numpy as np
import ml_dtypes

import concourse.mybir as mybir
import concourse.tile as tile
from concourse import bacc
from concourse.bass_utils import run_bass_kernel_spmd
from concourse.vector_clock import ScopedClock


class _FastExitTileContext(tile.TileContext):
    """TileContext whose exit skips the second all-engine barrier.

    The final barrier only orders the gpsimd semaphore-clears against the
    other engines' completion; NEFF execution completion already waits for
    every engine's last instruction, and the clears still run, so repeated
    executions stay correct. Saves ~2-3us of kernel tail.
    """

    def _drain_and_barrier(self, tick_clock, wait_clock):
        drain_inst = self.nc.sync.drain()
        wait_clock.add_sem_waits(
            drain_inst.ins, ScopedClock({None: tick_clock.global_clock})
        )
        self.nc.all_engine_barrier()
        popped = self.nc._tile_sem_poison_stack.pop()
        assert popped is self._sem_poison
        self.nc.clear_and_free_semaphores(list(self.sems.allocated().values()))


F32 = mybir.dt.float32
F16 = mybir.dt.float16
BF16 = mybir.dt.bfloat16
I16 = mybir.dt.int16

L = 4096          # sequence length (both q and k)
F = 64            # feature dim
NJ = 32           # q-blocks of 128 (slots per chunk)
CHW = 512         # k-chunk width
NCH = 8           # number of k-chunks

# DVE Schraudolph constants: i16 = round(s * SCHRAU_SCALE + SCHRAU_B)
SCHRAU_SCALE = float(128.0 * np.log2(np.e))   # 184.664...
SCHRAU_B = float(16256 - 6)                   # 127<<7 minus sawtooth centering

LAG = 8           # P@V emission lag in slots


def chunk_layout(c):
    """Per-chunk supertile list: each entry is a list of per-slot kinds.

    'A' slots feed one exact-exp ACTIVATE (contiguous leading cols of the
    tile); a trailing 'D' slot is the tile's last bank, consumed by the DVE
    Schraudolph op. Steady chunks: 9x[A,A,D] + [A,A,A] + [A,D] -> 10 D slots.
    Chunk 0 leads with small ACT groups to prime the pipeline; the last
    chunk mirrors so the tail drains fast.
    """
    m, a, ad = ["A", "A", "D"], ["A", "A", "A"], ["A", "D"]
    if c == 0:
        pat = [["A"], ["A"], ["A", "A"], ["A", "A"]] + [m] * 8 + [ad]
    elif c == NCH - 1:
        pat = [m] * 9 + [ad, ["A", "A"], ["A"]]
    else:
        pat = [m] * 9 + [a, ad]
    assert sum(len(t) for t in pat) == NJ
    return pat


def build_nc():
    nc = bacc.Bacc(None, target_bir_lowering=False)

    xqT = nc.dram_tensor("xqT", [128, L // 2], F16, kind="ExternalInput")
    ktil = nc.dram_tensor("ktil", [128, L], F16, kind="ExternalInput")
    vaug = nc.dram_tensor("vaug", [128, NJ * (F + 1)], BF16, kind="ExternalInput")
    # Per chunk c: cols [1024c, 1024c+512) = low-q-half partial sums,
    # [1024c+512, 1024c+1024) = high-q-half; host adds the halves.
    uout = nc.dram_tensor("uout", [F + 1, 2 * L], F32, kind="ExternalOutput")

    Exp = mybir.ActivationFunctionType.Exp
    Mult = mybir.AluOpType.mult
    Add = mybir.AluOpType.add

    # slot -> (kind, tile_key, offset_in_tile, tile_kinds, n_A_in_tile)
    layouts = {}
    for c in range(NCH):
        smap = {}
        s = 0
        for gi, kinds in enumerate(chunk_layout(c)):
            for off, kind in enumerate(kinds):
                smap[s] = (kind, (c, gi), off, kinds, kinds.count("A"))
                s += 1
        layouts[c] = smap

    with _FastExitTileContext(nc) as tc:
        with (
            tc.tile_pool(name="persist", bufs=1) as persist,
            tc.tile_pool(name="pt", bufs=6) as ptpool,
            tc.tile_pool(name="dpt", bufs=6) as dptpool,
            tc.tile_pool(name="utbf", bufs=2) as utbfpool,
            tc.tile_pool(name="ps_st", bufs=2, space="PSUM") as ps_st,
            tc.tile_pool(name="ps_ut", bufs=2, space="PSUM") as ps_ut,
        ):
            # Warm the ACT exp table while input DMAs are still in flight.
            warm_in = persist.tile([128, 8], F32)
            warm_out = persist.tile([128, 8], BF16)
            nc.vector.memset(warm_in[:], 0.0)
            nc.scalar.activation(warm_out[:], warm_in[:], Exp)

            # Split DMAs so the first iterations' inputs land early.
            xqT_sb = persist.tile([128, L // 2], F16)
            ktil_sb = persist.tile([128, L], F16)
            vaug_sb = persist.tile([128, NJ * (F + 1)], BF16)
            nc.sync.dma_start(xqT_sb[0:64, 0:128], xqT[0:64, 0:128])
            nc.sync.dma_start(ktil_sb[0:64, 0:CHW], ktil[0:64, 0:CHW])
            nc.sync.dma_start(xqT_sb[64:128, 0:128], xqT[64:128, 0:128])
            nc.sync.dma_start(ktil_sb[64:128, 0:CHW], ktil[64:128, 0:CHW])
            nc.sync.dma_start(vaug_sb[:, 0:2 * (F + 1)], vaug[:, 0:2 * (F + 1)])
            nc.sync.dma_start(xqT_sb[:, 128:], xqT[:, 128:])
            nc.sync.dma_start(vaug_sb[:, 2 * (F + 1):], vaug[:, 2 * (F + 1):])
            nc.sync.dma_start(ktil_sb[:, CHW:], ktil[:, CHW:])

            sts = {}    # tile_key -> psum supertile
            pta = {}    # tile_key -> ACT bf16 exp tile
            ptd = {}    # tile_key -> DVE i16 exp tile
            uts = {}    # chunk -> (utl, uth) psum accumulators

            def emit_scores(c, s):
                kind, key, off, kinds, n_a = layouts[c][s]
                if off == 0:
                    sts[key] = ps_st.tile([128, CHW * len(kinds)], F32,
                                          name="st", tag="st")
                st = sts[key]
                rh = 64 * (s % 2)
                qcols = slice(128 * (s // 2), 128 * (s // 2 + 1))
                kcols = slice(CHW * c, CHW * (c + 1))
                nc.tensor.matmul(st[:, CHW * off: CHW * (off + 1)],
                                 xqT_sb[rh:rh + 64, qcols],
                                 ktil_sb[rh:rh + 64, kcols],
                                 start=True, stop=True, tile_position=(rh, 0))
                if off == n_a - 1:
                    # all A slots of this tile are in flight; exp them as one
                    # ACTIVATE (contiguous leading cols).
                    pt = ptpool.tile([128, CHW * n_a], BF16, name="pt", tag="pt")
                    nc.scalar.activation(pt[:], st[:, 0: CHW * n_a], Exp)
                    pta[key] = pt
                if kind == "D":
                    dpt = dptpool.tile([128, CHW], I16, name="dpt", tag="dpt")
                    nc.vector.tensor_scalar(
                        dpt[:], st[:, CHW * off: CHW * (off + 1)],
                        SCHRAU_SCALE, SCHRAU_B, Mult, Add)
                    ptd[key] = dpt
                if off == len(kinds) - 1:
                    sts.pop(key)

            def emit_pav(c, s):
                if s == 0:
                    uts[c] = (ps_ut.tile([F + 1, CHW], F32, name="utl", tag="ut"),
                              ps_ut.tile([F + 1, CHW], F32, name="uth", tag="ut"))
                utl, uth = uts[c]
                kind, key, off, kinds, n_a = layouts[c][s]
                if kind == "A":
                    rhs = pta[key][:, CHW * off: CHW * (off + 1)]
                else:
                    rhs = ptd[key][:].bitcast(BF16)
                vsl = slice((F + 1) * s, (F + 1) * (s + 1))
                nc.tensor.matmul(utl[:], vaug_sb[0:64, vsl], rhs[0:64, :],
                                 start=(s == 0), stop=(s == NJ - 1),
                                 tile_position=(0, 0))
                nc.tensor.matmul(uth[:], vaug_sb[64:128, vsl], rhs[64:128, :],
                                 start=(s == 0), stop=(s == NJ - 1),
                                 tile_position=(64, 0))
                if kind == "A" and off == n_a - 1:
                    pta.pop(key)
                if kind == "D":
                    ptd.pop(key)

            def emit_epilogue(c):
                utl, uth = uts.pop(c)
                utbf = utbfpool.tile([F + 1, 2 * CHW], F32)
                nc.vector.tensor_copy(utbf[:, 0:CHW], utl[:])
                nc.vector.tensor_copy(utbf[:, CHW:], uth[:])
                nc.sync.dma_start(uout[:, 2 * CHW * c: 2 * CHW * (c + 1)],
                                  utbf[:])

            NTOT = NCH * NJ
            for gstep in range(NTOT + LAG):
                if gstep < NTOT:
                    emit_scores(gstep // NJ, gstep % NJ)
                if gstep >= LAG:
                    pc, ps = (gstep - LAG) // NJ, (gstep - LAG) % NJ
                    emit_pav(pc, ps)
                    if ps == NJ - 1:
                        emit_epilogue(pc)

    nc.compile()
    return nc


def host_pack(query_b, key_b, value_b, M, c):
    """Per-batch device-input packing (numpy, O(L*F))."""
    qT = query_b.T.reshape(F, L // 256, 2, 128)
    xqT = np.ascontiguousarray(                                       # [128, L/2]
        np.concatenate([qT[:, :, 0, :], qT[:, :, 1, :]], axis=0)
        .reshape(128, L // 2)).astype(np.float16)
    kt = (M @ key_b.T + c[:, None]).astype(np.float16)                # [64, L]
    ktil = np.ascontiguousarray(np.concatenate([kt, kt], axis=0))     # [128, L]
    v3 = value_b.reshape(NJ, 128, F).transpose(1, 0, 2)               # [128, NJ, F]
    vaug = np.ones((128, NJ, F + 1), np.float32)
    vaug[:, :, 0:F] = v3
    vaug_bf = vaug.reshape(128, NJ * (F + 1)).astype(ml_dtypes.bfloat16)
    return xqT, ktil, np.ascontiguousarray(vaug_bf)


def host_consts(wq, bq, wk, bk, wv, bv):
    wq64 = wq.astype(np.float64)
    M = (wq64.T @ wk.astype(np.float64)).astype(np.float32)
    c = (wq64.T @ bk.astype(np.float64)).astype(np.float32)
    return M, c


_NC = None


def kernel(**inputs):
    out, _ = run_kernel(inputs)
    return out


def run_kernel(inputs, **spmd_kwargs):
    global _NC
    if _NC is None:
        _NC = build_nc()

    query = np.asarray(inputs["query"], np.float32)
    key = np.asarray(inputs["key"], np.float32)
    value = np.asarray(inputs["value"], np.float32)
    wv = np.asarray(inputs["wv"], np.float32)
    bv = np.asarray(inputs["bv"], np.float32)
    M, c = host_consts(
        np.asarray(inputs["wq"], np.float32), np.asarray(inputs["bq"], np.float32),
        np.asarray(inputs["wk"], np.float32), np.asarray(inputs["bk"], np.float32),
        wv, bv)

    B = query.shape[0]
    in_maps = []
    for b in range(B):
        xqT, ktil, vaug = host_pack(query[b], key[b], value[b], M, c)
        in_maps.append({"xqT": xqT, "ktil": ktil, "vaug": vaug})
    res = run_bass_kernel_spmd(_NC, in_maps, core_ids=list(range(B)), **spmd_kwargs)
    outs = []
    for b in range(B):
        u2 = res.results[b]["uout"]             # [65, 2L]: per-chunk half-sums
        u2 = u2.reshape(F + 1, NCH, 2, CHW)
        u = (u2[:, :, 0, :] + u2[:, :, 1, :]).reshape(F + 1, L)
        ut = (u[0:F, :] / u[F:F + 1, :]).T      # [L, F] normalized attn @ value
        outs.append(ut @ wv.T + bv)             # host fp32 epilogue projection
    out = np.stack(outs).astype(np.float32)
    return out, res


# revision 9
# speedup vs baseline: 1.2200x; 1.1541x over previous
"""Trainium2 Bass kernel for nn_DotProductAttention (B=8, LQ=LK=4096, F=64).

Reference computation:
    q = query @ wq.T + bq ; k = key @ wk.T + bk ; v = value @ wv.T + bv
    scores = einsum('bkf,bqf->bkq', k, q)
    attn = softmax(scores, axis=-1)           # over q positions
    out = einsum('bkq,bqf->bkf', attn, v)

Strategy: batch b -> core b (8 cores, no cross-core communication).

Algebraic folding (host side, O(L*F) prep only -- all O(L^2) work on device):
    with M = wq^T wk, c = wq^T bk (per-k softmax-invariant term dropped):
        S^T[q,k] = query[q,:] @ ktil[:,k],   ktil = M @ key^T + c   (host)
    U^T = [value | 1]^T @ exp(S^T) accumulates in PSUM; its last row is the
    softmax denominator l. The tiny projection (U/l) @ wv.T + bv runs on host.

Device loop (per core): 8 k-chunks of 512 columns; per chunk sweep the 32
q-blocks (slots). The exp() of the 16.7M scores is the kernel bottleneck
(ACT engine: 1 elem/cycle/lane @1.2GHz = 109us alone), so exp is SPLIT:
  - ACT path: exact Exp ACTIVATEs over the leading banks of each PSUM
    supertile -> bf16 SBUF.
  - DVE path (~10 of 32 slots/chunk): the third bank of an "M" supertile is
    processed by one tensor_scalar: i16 = round(s * 128*log2(e) + B) from
    PSUM fp32 [128,512]; the int16 buffer IS the bf16 bit pattern of ~e^s
    (Schraudolph bit-trick exp2, ~3.4% max elem err; ~1.2e-2 output err at
    ~30% routing -- inside the 2e-2 budget). ScalarE/VectorE read different
    PSUM banks of the same tile in parallel (legal on TRN2).
P@V lags LAG slots behind scores: two concurrent 64-row-group matmuls
(tile_position row tiling) accumulate q-halves into utl/uth (separate PSUM
banks -- concurrent drains into one bank are fatal). Host adds the halves.
PSUM: 2x3 supertile banks + utl + uth = 8.
"""

import numpy as np
import ml_dtypes

import concourse.mybir as mybir
import concourse.tile as tile
from concourse import bacc
from concourse.bass_utils import run_bass_kernel_spmd
from concourse.vector_clock import ScopedClock


class _FastExitTileContext(tile.TileContext):
    """TileContext whose exit skips the second all-engine barrier.

    The final barrier only orders the gpsimd semaphore-clears against the
    other engines' completion; NEFF execution completion already waits for
    every engine's last instruction, and the clears still run, so repeated
    executions stay correct. Saves ~2-3us of kernel tail.
    """

    def _drain_and_barrier(self, tick_clock, wait_clock):
        drain_inst = self.nc.sync.drain()
        wait_clock.add_sem_waits(
            drain_inst.ins, ScopedClock({None: tick_clock.global_clock})
        )
        self.nc.all_engine_barrier()
        popped = self.nc._tile_sem_poison_stack.pop()
        assert popped is self._sem_poison
        self.nc.clear_and_free_semaphores(list(self.sems.allocated().values()))


F32 = mybir.dt.float32
F16 = mybir.dt.float16
BF16 = mybir.dt.bfloat16
I16 = mybir.dt.int16

L = 4096          # sequence length (both q and k)
F = 64            # feature dim
NJ = 32           # q-blocks of 128 (slots per chunk)
CHW = 512         # k-chunk width
NCH = 8           # number of k-chunks

# DVE Schraudolph constants: i16 = round(s * SCHRAU_SCALE + SCHRAU_B)
SCHRAU_SCALE = float(128.0 * np.log2(np.e))   # 184.664...
SCHRAU_B = float(16256 - 6)                   # 127<<7 minus sawtooth centering

LAG = 8           # P@V emission lag in slots


def chunk_layout(c):
    """Per-chunk supertile list: each entry is a list of per-slot kinds.

    'A' slots feed one exact-exp ACTIVATE (contiguous leading cols of the
    tile); a trailing 'D' slot is the tile's last bank, consumed by the DVE
    Schraudolph op. Steady chunks: 9x[A,A,D] + [A,A,A] + [A,D] -> 10 D slots.
    Chunk 0 leads with small ACT groups to prime the pipeline; the last
    chunk mirrors so the tail drains fast.
    """
    m, a, ad = ["D", "A", "A"], ["A", "A", "A"], ["D", "A"]
    if c == 0:
        pat = [["A"], ["A"], ["A", "A"], ["A", "A"]] + [m] * 8 + [ad]
    elif c == NCH - 1:
        pat = [m] * 9 + [ad, ["A", "A"], ["A"]]
    else:
        pat = [m] * 9 + [a, ad]
    assert sum(len(t) for t in pat) == NJ
    return pat


def build_nc():
    nc = bacc.Bacc(None, target_bir_lowering=False)

    xqT = nc.dram_tensor("xqT", [128, L // 2], F16, kind="ExternalInput")
    ktil = nc.dram_tensor("ktil", [128, L], F16, kind="ExternalInput")
    vaug = nc.dram_tensor("vaug", [128, NJ * (F + 1)], BF16, kind="ExternalInput")
    # Per chunk c: cols [1024c, 1024c+512) = low-q-half partial sums,
    # [1024c+512, 1024c+1024) = high-q-half; host adds the halves.
    uout = nc.dram_tensor("uout", [F + 1, 2 * L], F32, kind="ExternalOutput")

    Exp = mybir.ActivationFunctionType.Exp
    Mult = mybir.AluOpType.mult
    Add = mybir.AluOpType.add

    # slot -> (kind, tile_key, offset_in_tile, tile_kinds, a_start)
    layouts = {}
    for c in range(NCH):
        smap = {}
        s = 0
        for gi, kinds in enumerate(chunk_layout(c)):
            for off, kind in enumerate(kinds):
                smap[s] = (kind, (c, gi), off, kinds, kinds.index("A"))
                s += 1
        layouts[c] = smap

    with _FastExitTileContext(nc) as tc:
        with (
            tc.tile_pool(name="persist", bufs=1) as persist,
            tc.tile_pool(name="pt", bufs=6) as ptpool,
            tc.tile_pool(name="dpt", bufs=6) as dptpool,
            tc.tile_pool(name="utbf", bufs=2) as utbfpool,
            tc.tile_pool(name="ps_st", bufs=2, space="PSUM") as ps_st,
            tc.tile_pool(name="ps_ut", bufs=2, space="PSUM") as ps_ut,
        ):
            # Warm the ACT exp table while input DMAs are still in flight.
            warm_in = persist.tile([128, 8], F32)
            warm_out = persist.tile([128, 8], BF16)
            nc.vector.memset(warm_in[:], 0.0)
            nc.scalar.activation(warm_out[:], warm_in[:], Exp)

            # Split DMAs so the first iterations' inputs land early.
            xqT_sb = persist.tile([128, L // 2], F16)
            ktil_sb = persist.tile([128, L], F16)
            vaug_sb = persist.tile([128, NJ * (F + 1)], BF16)
            nc.sync.dma_start(xqT_sb[0:64, 0:128], xqT[0:64, 0:128])
            nc.sync.dma_start(ktil_sb[0:64, 0:CHW], ktil[0:64, 0:CHW])
            nc.sync.dma_start(xqT_sb[64:128, 0:128], xqT[64:128, 0:128])
            nc.sync.dma_start(ktil_sb[64:128, 0:CHW], ktil[64:128, 0:CHW])
            nc.sync.dma_start(vaug_sb[:, 0:2 * (F + 1)], vaug[:, 0:2 * (F + 1)])
            nc.sync.dma_start(xqT_sb[:, 128:], xqT[:, 128:])
            nc.sync.dma_start(vaug_sb[:, 2 * (F + 1):], vaug[:, 2 * (F + 1):])
            nc.sync.dma_start(ktil_sb[:, CHW:], ktil[:, CHW:])

            sts = {}    # tile_key -> psum supertile
            pta = {}    # tile_key -> ACT bf16 exp tile
            ptd = {}    # tile_key -> DVE i16 exp tile
            uts = {}    # chunk -> (utl, uth) psum accumulators

            def emit_scores(c, s):
                kind, key, off, kinds, a0 = layouts[c][s]
                if off == 0:
                    sts[key] = ps_st.tile([128, CHW * len(kinds)], F32,
                                          name="st", tag="st")
                st = sts[key]
                rh = 64 * (s % 2)
                qcols = slice(128 * (s // 2), 128 * (s // 2 + 1))
                kcols = slice(CHW * c, CHW * (c + 1))
                nc.tensor.matmul(st[:, CHW * off: CHW * (off + 1)],
                                 xqT_sb[rh:rh + 64, qcols],
                                 ktil_sb[rh:rh + 64, kcols],
                                 start=True, stop=True, tile_position=(rh, 0))
                if kind == "D":
                    # D slot leads the tile so the DVE read releases the PSUM
                    # buffer early (keeps the WAR gate on the ACTIVATE).
                    dpt = dptpool.tile([128, CHW], I16, name="dpt", tag="dpt")
                    nc.vector.tensor_scalar(
                        dpt[:], st[:, CHW * off: CHW * (off + 1)],
                        SCHRAU_SCALE, SCHRAU_B, Mult, Add)
                    ptd[key] = dpt
                if off == len(kinds) - 1:
                    # all A slots in flight; exp them as one contiguous
                    # ACTIVATE over cols [a0*CHW, end).
                    n_a = len(kinds) - a0
                    pt = ptpool.tile([128, CHW * n_a], BF16, name="pt", tag="pt")
                    nc.scalar.activation(pt[:], st[:, CHW * a0:], Exp)
                    pta[key] = pt
                    sts.pop(key)

            def emit_pav(c, s):
                if s == 0:
                    uts[c] = (ps_ut.tile([F + 1, CHW], F32, name="utl", tag="ut"),
                              ps_ut.tile([F + 1, CHW], F32, name="uth", tag="ut"))
                utl, uth = uts[c]
                kind, key, off, kinds, a0 = layouts[c][s]
                if kind == "A":
                    rhs = pta[key][:, CHW * (off - a0): CHW * (off - a0 + 1)]
                else:
                    rhs = ptd[key][:].bitcast(BF16)
                vsl = slice((F + 1) * s, (F + 1) * (s + 1))
                nc.tensor.matmul(utl[:], vaug_sb[0:64, vsl], rhs[0:64, :],
                                 start=(s == 0), stop=(s == NJ - 1),
                                 tile_position=(0, 0))
                nc.tensor.matmul(uth[:], vaug_sb[64:128, vsl], rhs[64:128, :],
                                 start=(s == 0), stop=(s == NJ - 1),
                                 tile_position=(64, 0))
                if kind == "A" and off == len(kinds) - 1:
                    pta.pop(key)
                if kind == "D":
                    ptd.pop(key)

            def emit_epilogue(c):
                utl, uth = uts.pop(c)
                utbf = utbfpool.tile([F + 1, 2 * CHW], F32)
                nc.vector.tensor_copy(utbf[:, 0:CHW], utl[:])
                nc.vector.tensor_copy(utbf[:, CHW:], uth[:])
                nc.sync.dma_start(uout[:, 2 * CHW * c: 2 * CHW * (c + 1)],
                                  utbf[:])

            NTOT = NCH * NJ
            for gstep in range(NTOT + LAG):
                if gstep < NTOT:
                    emit_scores(gstep // NJ, gstep % NJ)
                if gstep >= LAG:
                    pc, ps = (gstep - LAG) // NJ, (gstep - LAG) % NJ
                    emit_pav(pc, ps)
                    if ps == NJ - 1:
                        emit_epilogue(pc)

    nc.compile()
    return nc


def host_pack(query_b, key_b, value_b, M, c):
    """Per-batch device-input packing (numpy, O(L*F))."""
    qT = query_b.T.reshape(F, L // 256, 2, 128)
    xqT = np.ascontiguousarray(                                       # [128, L/2]
        np.concatenate([qT[:, :, 0, :], qT[:, :, 1, :]], axis=0)
        .reshape(128, L // 2)).astype(np.float16)
    kt = (M @ key_b.T + c[:, None]).astype(np.float16)                # [64, L]
    ktil = np.ascontiguousarray(np.concatenate([kt, kt], axis=0))     # [128, L]
    v3 = value_b.reshape(NJ, 128, F).transpose(1, 0, 2)               # [128, NJ, F]
    vaug = np.ones((128, NJ, F + 1), np.float32)
    vaug[:, :, 0:F] = v3
    vaug_bf = vaug.reshape(128, NJ * (F + 1)).astype(ml_dtypes.bfloat16)
    return xqT, ktil, np.ascontiguousarray(vaug_bf)


def host_consts(wq, bq, wk, bk, wv, bv):
    wq64 = wq.astype(np.float64)
    M = (wq64.T @ wk.astype(np.float64)).astype(np.float32)
    c = (wq64.T @ bk.astype(np.float64)).astype(np.float32)
    return M, c


_NC = None


def kernel(**inputs):
    out, _ = run_kernel(inputs)
    return out


def run_kernel(inputs, **spmd_kwargs):
    global _NC
    if _NC is None:
        _NC = build_nc()

    query = np.asarray(inputs["query"], np.float32)
    key = np.asarray(inputs["key"], np.float32)
    value = np.asarray(inputs["value"], np.float32)
    wv = np.asarray(inputs["wv"], np.float32)
    bv = np.asarray(inputs["bv"], np.float32)
    M, c = host_consts(
        np.asarray(inputs["wq"], np.float32), np.asarray(inputs["bq"], np.float32),
        np.asarray(inputs["wk"], np.float32), np.asarray(inputs["bk"], np.float32),
        wv, bv)

    B = query.shape[0]
    in_maps = []
    for b in range(B):
        xqT, ktil, vaug = host_pack(query[b], key[b], value[b], M, c)
        in_maps.append({"xqT": xqT, "ktil": ktil, "vaug": vaug})
    res = run_bass_kernel_spmd(_NC, in_maps, core_ids=list(range(B)), **spmd_kwargs)
    outs = []
    for b in range(B):
        u2 = res.results[b]["uout"]             # [65, 2L]: per-chunk half-sums
        u2 = u2.reshape(F + 1, NCH, 2, CHW)
        u = (u2[:, :, 0, :] + u2[:, :, 1, :]).reshape(F + 1, L)
        ut = (u[0:F, :] / u[F:F + 1, :]).T      # [L, F] normalized attn @ value
        outs.append(ut @ wv.T + bv)             # host fp32 epilogue projection
    out = np.stack(outs).astype(np.float32)
    return out, res


# revision 10
# speedup vs baseline: 1.2313x; 1.0093x over previous
"""Trainium2 Bass kernel for nn_DotProductAttention (B=8, LQ=LK=4096, F=64).

Reference computation:
    q = query @ wq.T + bq ; k = key @ wk.T + bk ; v = value @ wv.T + bv
    scores = einsum('bkf,bqf->bkq', k, q)
    attn = softmax(scores, axis=-1)           # over q positions
    out = einsum('bkq,bqf->bkf', attn, v)

Strategy: batch b -> core b (8 cores, no cross-core communication).

Algebraic folding (host side, O(L*F) prep only -- all O(L^2) work on device):
    with M = wq^T wk, c = wq^T bk (per-k softmax-invariant term dropped):
        S^T[q,k] = query[q,:] @ ktil[:,k],   ktil = M @ key^T + c   (host)
    U^T = [value | 1]^T @ exp(S^T) accumulates in PSUM; its last row is the
    softmax denominator l. The tiny projection (U/l) @ wv.T + bv runs on host.

Device loop (per core): 8 k-chunks of 512 columns; per chunk sweep the 32
q-blocks (slots). The exp() of the 16.7M scores is the kernel bottleneck
(ACT engine: 1 elem/cycle/lane @1.2GHz = 109us alone), so exp is SPLIT:
  - ACT path: exact Exp ACTIVATEs over the leading banks of each PSUM
    supertile -> bf16 SBUF.
  - DVE path (~10 of 32 slots/chunk): the third bank of an "M" supertile is
    processed by one tensor_scalar: i16 = round(s * 128*log2(e) + B) from
    PSUM fp32 [128,512]; the int16 buffer IS the bf16 bit pattern of ~e^s
    (Schraudolph bit-trick exp2, ~3.4% max elem err; ~1.2e-2 output err at
    ~30% routing -- inside the 2e-2 budget). ScalarE/VectorE read different
    PSUM banks of the same tile in parallel (legal on TRN2).
P@V lags LAG slots behind scores: two concurrent 64-row-group matmuls
(tile_position row tiling) accumulate q-halves into utl/uth (separate PSUM
banks -- concurrent drains into one bank are fatal). Host adds the halves.
PSUM: 2x3 supertile banks + utl + uth = 8.
"""

import numpy as np
import ml_dtypes

import concourse.mybir as mybir
import concourse.tile as tile
from concourse import bacc
from concourse.bass_utils import run_bass_kernel_spmd
from concourse.vector_clock import ScopedClock


class _FastExitTileContext(tile.TileContext):
    """TileContext whose exit skips the second all-engine barrier.

    The final barrier only orders the gpsimd semaphore-clears against the
    other engines' completion; NEFF execution completion already waits for
    every engine's last instruction, and the clears still run, so repeated
    executions stay correct. Saves ~2-3us of kernel tail.
    """

    def _drain_and_barrier(self, tick_clock, wait_clock):
        drain_inst = self.nc.sync.drain()
        wait_clock.add_sem_waits(
            drain_inst.ins, ScopedClock({None: tick_clock.global_clock})
        )
        self.nc.all_engine_barrier()
        popped = self.nc._tile_sem_poison_stack.pop()
        assert popped is self._sem_poison
        self.nc.clear_and_free_semaphores(list(self.sems.allocated().values()))


F32 = mybir.dt.float32
F16 = mybir.dt.float16
BF16 = mybir.dt.bfloat16
I16 = mybir.dt.int16

L = 4096          # sequence length (both q and k)
F = 64            # feature dim
NJ = 32           # q-blocks of 128 (slots per chunk)
CHW = 512         # k-chunk width
NCH = 8           # number of k-chunks

# DVE Schraudolph constants: i16 = round(s * SCHRAU_SCALE + SCHRAU_B)
SCHRAU_SCALE = float(128.0 * np.log2(np.e))   # 184.664...
SCHRAU_B = float(16256 - 6)                   # 127<<7 minus sawtooth centering

LAG = 8           # P@V emission lag in slots


def chunk_layout(c):
    """Per-chunk supertile list: each entry is a list of per-slot kinds.

    'A' slots feed one exact-exp ACTIVATE (contiguous leading cols of the
    tile); a trailing 'D' slot is the tile's last bank, consumed by the DVE
    Schraudolph op. Steady chunks: 9x[A,A,D] + [A,A,A] + [A,D] -> 10 D slots.
    Chunk 0 leads with small ACT groups to prime the pipeline; the last
    chunk mirrors so the tail drains fast.
    """
    m, a, ad = ["D", "A", "A"], ["A", "A", "A"], ["D", "A"]
    if c == 0:
        pat = [["A"], ["A"], ["A", "A"], ["A", "A"]] + [m] * 8 + [ad]
    elif c == NCH - 1:
        pat = [m] * 9 + [ad, ["A", "A"], ["A"]]
    else:
        pat = [m] * 9 + [a, ad]
    assert sum(len(t) for t in pat) == NJ
    return pat


def build_nc():
    nc = bacc.Bacc(None, target_bir_lowering=False)

    xqT = nc.dram_tensor("xqT", [128, L // 2], F16, kind="ExternalInput")
    ktil = nc.dram_tensor("ktil", [128, L], F16, kind="ExternalInput")
    vaug = nc.dram_tensor("vaug", [128, NJ * (F + 1)], BF16, kind="ExternalInput")
    # Per chunk c: cols [1024c, 1024c+512) = low-q-half partial sums,
    # [1024c+512, 1024c+1024) = high-q-half; host adds the halves.
    uout = nc.dram_tensor("uout", [F + 1, 2 * L], F32, kind="ExternalOutput")

    Exp = mybir.ActivationFunctionType.Exp
    Mult = mybir.AluOpType.mult
    Add = mybir.AluOpType.add

    # slot -> (kind, tile_key, offset_in_tile, tile_kinds, a_start)
    layouts = {}
    for c in range(NCH):
        smap = {}
        s = 0
        for gi, kinds in enumerate(chunk_layout(c)):
            for off, kind in enumerate(kinds):
                smap[s] = (kind, (c, gi), off, kinds, kinds.index("A"))
                s += 1
        layouts[c] = smap

    with _FastExitTileContext(nc) as tc:
        with (
            tc.tile_pool(name="persist", bufs=1) as persist,
            tc.tile_pool(name="pt", bufs=6) as ptpool,
            tc.tile_pool(name="dpt", bufs=6) as dptpool,
            tc.tile_pool(name="utbf", bufs=2) as utbfpool,
            tc.tile_pool(name="ps_st", bufs=2, space="PSUM") as ps_st,
            tc.tile_pool(name="ps_ut", bufs=2, space="PSUM") as ps_ut,
        ):
            # Warm the ACT exp table while input DMAs are still in flight.
            warm_in = persist.tile([128, 8], F32)
            warm_out = persist.tile([128, 8], BF16)
            nc.vector.memset(warm_in[:], 0.0)
            nc.scalar.activation(warm_out[:], warm_in[:], Exp)

            # Split DMAs so the first iterations' inputs land early.
            xqT_sb = persist.tile([128, L // 2], F16)
            ktil_sb = persist.tile([128, L], F16)
            vaug_sb = persist.tile([128, NJ * (F + 1)], BF16)
            nc.sync.dma_start(xqT_sb[0:64, 0:128], xqT[0:64, 0:128])
            nc.sync.dma_start(ktil_sb[0:64, 0:CHW], ktil[0:64, 0:CHW])
            nc.sync.dma_start(xqT_sb[64:128, 0:128], xqT[64:128, 0:128])
            nc.sync.dma_start(ktil_sb[64:128, 0:CHW], ktil[64:128, 0:CHW])
            nc.sync.dma_start(vaug_sb[:, 0:2 * (F + 1)], vaug[:, 0:2 * (F + 1)])
            nc.sync.dma_start(xqT_sb[:, 128:], xqT[:, 128:])
            nc.sync.dma_start(vaug_sb[:, 2 * (F + 1):], vaug[:, 2 * (F + 1):])
            nc.sync.dma_start(ktil_sb[:, CHW:], ktil[:, CHW:])

            sts = {}    # tile_key -> psum supertile
            pta = {}    # tile_key -> ACT bf16 exp tile
            ptd = {}    # tile_key -> DVE i16 exp tile
            uts = {}    # chunk -> (utl, uth) psum accumulators

            def emit_scores(c, s):
                kind, key, off, kinds, a0 = layouts[c][s]
                if off == 0:
                    sts[key] = ps_st.tile([128, CHW * len(kinds)], F32,
                                          name="st", tag="st")
                st = sts[key]
                rh = 64 * (s % 2)
                qcols = slice(128 * (s // 2), 128 * (s // 2 + 1))
                kcols = slice(CHW * c, CHW * (c + 1))
                nc.tensor.matmul(st[:, CHW * off: CHW * (off + 1)],
                                 xqT_sb[rh:rh + 64, qcols],
                                 ktil_sb[rh:rh + 64, kcols],
                                 start=True, stop=True, tile_position=(rh, 0))
                if kind == "D":
                    # D slot leads the tile so the DVE read releases the PSUM
                    # buffer early (keeps the WAR gate on the ACTIVATE).
                    dpt = dptpool.tile([128, CHW], I16, name="dpt", tag="dpt")
                    nc.vector.tensor_scalar(
                        dpt[:], st[:, CHW * off: CHW * (off + 1)],
                        SCHRAU_SCALE, SCHRAU_B, Mult, Add)
                    ptd[key] = dpt
                if off == len(kinds) - 1:
                    # all A slots in flight; exp them as one contiguous
                    # ACTIVATE over cols [a0*CHW, end).
                    n_a = len(kinds) - a0
                    pt = ptpool.tile([128, CHW * n_a], BF16, name="pt", tag="pt")
                    nc.scalar.activation(pt[:], st[:, CHW * a0:], Exp)
                    pta[key] = pt
                    sts.pop(key)

            def emit_pav(c, s):
                if s == 0:
                    uts[c] = (ps_ut.tile([F + 1, CHW], F32, name="utl", tag="ut"),
                              ps_ut.tile([F + 1, CHW], F32, name="uth", tag="ut"))
                utl, uth = uts[c]
                kind, key, off, kinds, a0 = layouts[c][s]
                if kind == "A":
                    rhs = pta[key][:, CHW * (off - a0): CHW * (off - a0 + 1)]
                else:
                    rhs = ptd[key][:].bitcast(BF16)
                vsl = slice((F + 1) * s, (F + 1) * (s + 1))
                # Emit the half OPPOSITE to this slot's scores row-group
                # first: the PE stream then alternates h0/h64 on every
                # adjacent matmul, so row-tiled pairing is continuous.
                halves = [
                    (utl, vaug_sb[0:64, vsl], rhs[0:64, :], (0, 0)),
                    (uth, vaug_sb[64:128, vsl], rhs[64:128, :], (64, 0)),
                ]
                for out, w, r, tp in (halves if s % 2 else halves[::-1]):
                    nc.tensor.matmul(out[:], w, r,
                                     start=(s == 0), stop=(s == NJ - 1),
                                     tile_position=tp)
                if kind == "A" and off == len(kinds) - 1:
                    pta.pop(key)
                if kind == "D":
                    ptd.pop(key)

            def emit_epilogue(c):
                utl, uth = uts.pop(c)
                utbf = utbfpool.tile([F + 1, 2 * CHW], F32)
                nc.vector.tensor_copy(utbf[:, 0:CHW], utl[:])
                nc.vector.tensor_copy(utbf[:, CHW:], uth[:])
                nc.sync.dma_start(uout[:, 2 * CHW * c: 2 * CHW * (c + 1)],
                                  utbf[:])

            NTOT = NCH * NJ
            for gstep in range(NTOT + LAG):
                if gstep < NTOT:
                    emit_scores(gstep // NJ, gstep % NJ)
                if gstep >= LAG:
                    pc, ps = (gstep - LAG) // NJ, (gstep - LAG) % NJ
                    emit_pav(pc, ps)
                    if ps == NJ - 1:
                        emit_epilogue(pc)

    nc.compile()
    return nc


def host_pack(query_b, key_b, value_b, M, c):
    """Per-batch device-input packing (numpy, O(L*F))."""
    qT = query_b.T.reshape(F, L // 256, 2, 128)
    xqT = np.ascontiguousarray(                                       # [128, L/2]
        np.concatenate([qT[:, :, 0, :], qT[:, :, 1, :]], axis=0)
        .reshape(128, L // 2)).astype(np.float16)
    kt = (M @ key_b.T + c[:, None]).astype(np.float16)                # [64, L]
    ktil = np.ascontiguousarray(np.concatenate([kt, kt], axis=0))     # [128, L]
    v3 = value_b.reshape(NJ, 128, F).transpose(1, 0, 2)               # [128, NJ, F]
    vaug = np.ones((128, NJ, F + 1), np.float32)
    vaug[:, :, 0:F] = v3
    vaug_bf = vaug.reshape(128, NJ * (F + 1)).astype(ml_dtypes.bfloat16)
    return xqT, ktil, np.ascontiguousarray(vaug_bf)


def host_consts(wq, bq, wk, bk, wv, bv):
    wq64 = wq.astype(np.float64)
    M = (wq64.T @ wk.astype(np.float64)).astype(np.float32)
    c = (wq64.T @ bk.astype(np.float64)).astype(np.float32)
    return M, c


_NC = None


def kernel(**inputs):
    out, _ = run_kernel(inputs)
    return out


def run_kernel(inputs, **spmd_kwargs):
    global _NC
    if _NC is None:
        _NC = build_nc()

    query = np.asarray(inputs["query"], np.float32)
    key = np.asarray(inputs["key"], np.float32)
    value = np.asarray(inputs["value"], np.float32)
    wv = np.asarray(inputs["wv"], np.float32)
    bv = np.asarray(inputs["bv"], np.float32)
    M, c = host_consts(
        np.asarray(inputs["wq"], np.float32), np.asarray(inputs["bq"], np.float32),
        np.asarray(inputs["wk"], np.float32), np.asarray(inputs["bk"], np.float32),
        wv, bv)

    B = query.shape[0]
    in_maps = []
    for b in range(B):
        xqT, ktil, vaug = host_pack(query[b], key[b], value[b], M, c)
        in_maps.append({"xqT": xqT, "ktil": ktil, "vaug": vaug})
    res = run_bass_kernel_spmd(_NC, in_maps, core_ids=list(range(B)), **spmd_kwargs)
    outs = []
    for b in range(B):
        u2 = res.results[b]["uout"]             # [65, 2L]: per-chunk half-sums
        u2 = u2.reshape(F + 1, NCH, 2, CHW)
        u = (u2[:, :, 0, :] + u2[:, :, 1, :]).reshape(F + 1, L)
        ut = (u[0:F, :] / u[F:F + 1, :]).T      # [L, F] normalized attn @ value
        outs.append(ut @ wv.T + bv)             # host fp32 epilogue projection
    out = np.stack(outs).astype(np.float32)
    return out, res


# revision 12
# speedup vs baseline: 1.5532x; 1.2614x over previous
"""Trainium2 Bass kernel for nn_DotProductAttention (B=8, LQ=LK=4096, F=64).

Reference computation:
    q = query @ wq.T + bq ; k = key @ wk.T + bk ; v = value @ wv.T + bv
    scores = einsum('bkf,bqf->bkq', k, q)
    attn = softmax(scores, axis=-1)           # over q positions
    out = einsum('bkq,bqf->bkf', attn, v)

Strategy: batch b -> core b (8 cores, no cross-core communication).

Algebraic folding (host side, O(L*F) prep only -- all O(L^2) work on device):
    with M = wq^T wk, c = wq^T bk (per-k softmax-invariant term dropped):
        S^T[q,k] = query[q,:] @ ktil[:,k],   ktil = M @ key^T + c   (host)
    U^T = [value | 1]^T @ exp(S^T) accumulates in PSUM; its last row is the
    softmax denominator l. The tiny projection (U/l) @ wv.T + bv runs on host.

Device loop (per core): 8 k-chunks of 512 columns; per chunk sweep the 32
q-blocks (slots). The exp() of the 16.7M scores is the kernel bottleneck
(ACT engine: 1 elem/cycle/lane @1.2GHz = 109us alone), so exp is SPLIT:
  - ACT path: exact Exp ACTIVATEs over the leading banks of each PSUM
    supertile -> bf16 SBUF.
  - DVE path (~10 of 32 slots/chunk): the third bank of an "M" supertile is
    processed by one tensor_scalar: i16 = round(s * 128*log2(e) + B) from
    PSUM fp32 [128,512]; the int16 buffer IS the bf16 bit pattern of ~e^s
    (Schraudolph bit-trick exp2, ~3.4% max elem err; ~1.2e-2 output err at
    ~30% routing -- inside the 2e-2 budget). ScalarE/VectorE read different
    PSUM banks of the same tile in parallel (legal on TRN2).
P@V lags LAG slots behind scores: two concurrent 64-row-group matmuls
(tile_position row tiling) accumulate q-halves into utl/uth (separate PSUM
banks -- concurrent drains into one bank are fatal). Host adds the halves.
PSUM: 2x3 supertile banks + utl + uth = 8.
"""

import numpy as np
import ml_dtypes

import concourse.mybir as mybir
import concourse.tile as tile
from concourse import bacc
from concourse.bass_utils import run_bass_kernel_spmd
from concourse.vector_clock import ScopedClock


class _FastExitTileContext(tile.TileContext):
    """TileContext whose exit skips the second all-engine barrier.

    The final barrier only orders the gpsimd semaphore-clears against the
    other engines' completion; NEFF execution completion already waits for
    every engine's last instruction, and the clears still run, so repeated
    executions stay correct. Saves ~2-3us of kernel tail.
    """

    def _drain_and_barrier(self, tick_clock, wait_clock):
        drain_inst = self.nc.sync.drain()
        wait_clock.add_sem_waits(
            drain_inst.ins, ScopedClock({None: tick_clock.global_clock})
        )
        self.nc.all_engine_barrier()
        popped = self.nc._tile_sem_poison_stack.pop()
        assert popped is self._sem_poison
        self.nc.clear_and_free_semaphores(list(self.sems.allocated().values()))


F32 = mybir.dt.float32
F16 = mybir.dt.float16
BF16 = mybir.dt.bfloat16
I16 = mybir.dt.int16

L = 4096          # sequence length (both q and k)
F = 64            # feature dim
NJ = 32           # q-blocks of 128 (slots per chunk)
CHW = 512         # k-chunk width
NCH = 8           # number of k-chunks

# DVE Schraudolph constants: i16 = round(s * SCHRAU_SCALE + SCHRAU_B)
SCHRAU_SCALE = float(128.0 * np.log2(np.e))   # 184.664...
SCHRAU_B = float(16256 - 6)                   # 127<<7 minus sawtooth centering

LAG = 8           # P@V emission lag in slots


def chunk_layout(c):
    """Per-chunk supertile list: each entry is a list of per-slot kinds.

    'A' slots feed one exact-exp ACTIVATE (contiguous leading cols of the
    tile); a trailing 'D' slot is the tile's last bank, consumed by the DVE
    Schraudolph op. Steady chunks: 9x[A,A,D] + [A,A,A] + [A,D] -> 10 D slots.
    Chunk 0 leads with small ACT groups to prime the pipeline; the last
    chunk mirrors so the tail drains fast.
    """
    m, ad = ["D", "A", "A"], ["D", "A"]
    if c == 0:
        pat = [["A"], ["A"], ["A", "A"], ["A", "A"]] + [m] * 8 + [ad]
    elif c == NCH - 1:
        pat = [m] * 9 + [ad, ["A", "A"], ["A"]]
    else:
        pat = [m] * 10 + [["A", "A"]]
    assert sum(len(t) for t in pat) == NJ
    return pat


def build_nc():
    nc = bacc.Bacc(None, target_bir_lowering=False)

    xqT = nc.dram_tensor("xqT", [128, L // 2], F16, kind="ExternalInput")
    ktil = nc.dram_tensor("ktil", [128, L], F16, kind="ExternalInput")
    vaug = nc.dram_tensor("vaug", [128, NJ * (F + 1)], BF16, kind="ExternalInput")
    # Per chunk c: cols [1024c, 1024c+512) = low-q-half partial sums,
    # [1024c+512, 1024c+1024) = high-q-half; host adds the halves.
    uout = nc.dram_tensor("uout", [F + 1, 2 * L], F32, kind="ExternalOutput")

    Exp = mybir.ActivationFunctionType.Exp
    Mult = mybir.AluOpType.mult
    Add = mybir.AluOpType.add

    # slot -> (kind, tile_key, offset_in_tile, tile_kinds, a_start)
    layouts = {}
    for c in range(NCH):
        smap = {}
        s = 0
        for gi, kinds in enumerate(chunk_layout(c)):
            for off, kind in enumerate(kinds):
                smap[s] = (kind, (c, gi), off, kinds, kinds.index("A"))
                s += 1
        layouts[c] = smap

    with _FastExitTileContext(nc) as tc:
        with (
            tc.tile_pool(name="persist", bufs=1) as persist,
            tc.tile_pool(name="pt", bufs=6) as ptpool,
            tc.tile_pool(name="dpt", bufs=6) as dptpool,
            tc.tile_pool(name="utbf", bufs=2) as utbfpool,
            tc.tile_pool(name="ps_st", bufs=2, space="PSUM") as ps_st,
            tc.tile_pool(name="ps_ut", bufs=2, space="PSUM") as ps_ut,
        ):
            # Warm the ACT exp table while input DMAs are still in flight.
            warm_in = persist.tile([128, 8], F32)
            warm_out = persist.tile([128, 8], BF16)
            nc.vector.memset(warm_in[:], 0.0)
            nc.scalar.activation(warm_out[:], warm_in[:], Exp)

            # Split DMAs so the first iterations' inputs land early.
            xqT_sb = persist.tile([128, L // 2], F16)
            ktil_sb = persist.tile([128, L], F16)
            vaug_sb = persist.tile([128, NJ * (F + 1)], BF16)
            nc.sync.dma_start(xqT_sb[0:64, 0:128], xqT[0:64, 0:128])
            nc.sync.dma_start(ktil_sb[0:64, 0:CHW], ktil[0:64, 0:CHW])
            nc.sync.dma_start(xqT_sb[64:128, 0:128], xqT[64:128, 0:128])
            nc.sync.dma_start(ktil_sb[64:128, 0:CHW], ktil[64:128, 0:CHW])
            nc.sync.dma_start(vaug_sb[:, 0:2 * (F + 1)], vaug[:, 0:2 * (F + 1)])
            nc.sync.dma_start(xqT_sb[:, 128:], xqT[:, 128:])
            nc.sync.dma_start(vaug_sb[:, 2 * (F + 1):], vaug[:, 2 * (F + 1):])
            nc.sync.dma_start(ktil_sb[:, CHW:], ktil[:, CHW:])

            sts = {}    # tile_key -> psum supertile
            pta = {}    # tile_key -> ACT bf16 exp tile
            ptd = {}    # tile_key -> DVE i16 exp tile
            uts = {}    # chunk -> (utl, uth) psum accumulators

            def emit_scores(c, s):
                kind, key, off, kinds, a0 = layouts[c][s]
                n_a = len(kinds) - a0
                if off == 0:
                    # The D slot gets its OWN tile (separate tag): ScalarE and
                    # VectorE then never read the same tile, avoiding Tile's
                    # conservative cross-engine read ordering (ACT stalling on
                    # the DVE TS). Banks/buffer budget unchanged: 2x(1+2).
                    std = (ps_st.tile([128, CHW], F32, name="std", tag="stD")
                           if a0 > 0 else None)
                    sta = ps_st.tile([128, CHW * n_a], F32, name="sta", tag="stA")
                    sts[key] = (std, sta)
                std, sta = sts[key]
                rh = 64 * (s % 2)
                qcols = slice(128 * (s // 2), 128 * (s // 2 + 1))
                kcols = slice(CHW * c, CHW * (c + 1))
                dst = (std[:, :] if kind == "D"
                       else sta[:, CHW * (off - a0): CHW * (off - a0 + 1)])
                nc.tensor.matmul(dst,
                                 xqT_sb[rh:rh + 64, qcols],
                                 ktil_sb[rh:rh + 64, kcols],
                                 start=True, stop=True, tile_position=(rh, 0))
                if kind == "D":
                    dpt = dptpool.tile([128, CHW], I16, name="dpt", tag="dpt")
                    nc.vector.tensor_scalar(
                        dpt[:], std[:, :],
                        SCHRAU_SCALE, SCHRAU_B, Mult, Add)
                    ptd[key] = dpt
                if off == len(kinds) - 1:
                    pt = ptpool.tile([128, CHW * n_a], BF16, name="pt", tag="pt")
                    nc.scalar.activation(pt[:], sta[:, :], Exp)
                    pta[key] = pt
                    sts.pop(key)

            def emit_pav(c, s):
                if s == 0:
                    uts[c] = (ps_ut.tile([F + 1, CHW], F32, name="utl", tag="ut"),
                              ps_ut.tile([F + 1, CHW], F32, name="uth", tag="ut"))
                utl, uth = uts[c]
                kind, key, off, kinds, a0 = layouts[c][s]
                if kind == "A":
                    rhs = pta[key][:, CHW * (off - a0): CHW * (off - a0 + 1)]
                else:
                    rhs = ptd[key][:].bitcast(BF16)
                vsl = slice((F + 1) * s, (F + 1) * (s + 1))
                # Emit the half OPPOSITE to this slot's scores row-group
                # first: the PE stream then alternates h0/h64 on every
                # adjacent matmul, so row-tiled pairing is continuous.
                halves = [
                    (utl, vaug_sb[0:64, vsl], rhs[0:64, :], (0, 0)),
                    (uth, vaug_sb[64:128, vsl], rhs[64:128, :], (64, 0)),
                ]
                for out, w, r, tp in (halves if s % 2 else halves[::-1]):
                    nc.tensor.matmul(out[:], w, r,
                                     start=(s == 0), stop=(s == NJ - 1),
                                     tile_position=tp)
                if kind == "A" and off == len(kinds) - 1:
                    pta.pop(key)
                if kind == "D":
                    ptd.pop(key)

            def emit_epilogue(c):
                utl, uth = uts.pop(c)
                utbf = utbfpool.tile([F + 1, 2 * CHW], F32)
                nc.vector.tensor_copy(utbf[:, 0:CHW], utl[:])
                nc.vector.tensor_copy(utbf[:, CHW:], uth[:])
                nc.sync.dma_start(uout[:, 2 * CHW * c: 2 * CHW * (c + 1)],
                                  utbf[:])

            NTOT = NCH * NJ
            for gstep in range(NTOT + LAG):
                if gstep < NTOT:
                    emit_scores(gstep // NJ, gstep % NJ)
                if gstep >= LAG:
                    pc, ps = (gstep - LAG) // NJ, (gstep - LAG) % NJ
                    emit_pav(pc, ps)
                    if ps == NJ - 1:
                        emit_epilogue(pc)

    nc.compile()
    return nc


def host_pack(query_b, key_b, value_b, M, c):
    """Per-batch device-input packing (numpy, O(L*F))."""
    qT = query_b.T.reshape(F, L // 256, 2, 128)
    xqT = np.ascontiguousarray(                                       # [128, L/2]
        np.concatenate([qT[:, :, 0, :], qT[:, :, 1, :]], axis=0)
        .reshape(128, L // 2)).astype(np.float16)
    kt = (M @ key_b.T + c[:, None]).astype(np.float16)                # [64, L]
    ktil = np.ascontiguousarray(np.concatenate([kt, kt], axis=0))     # [128, L]
    v3 = value_b.reshape(NJ, 128, F).transpose(1, 0, 2)               # [128, NJ, F]
    vaug = np.ones((128, NJ, F + 1), np.float32)
    vaug[:, :, 0:F] = v3
    vaug_bf = vaug.reshape(128, NJ * (F + 1)).astype(ml_dtypes.bfloat16)
    return xqT, ktil, np.ascontiguousarray(vaug_bf)


def host_consts(wq, bq, wk, bk, wv, bv):
    wq64 = wq.astype(np.float64)
    M = (wq64.T @ wk.astype(np.float64)).astype(np.float32)
    c = (wq64.T @ bk.astype(np.float64)).astype(np.float32)
    return M, c


_NC = None


def kernel(**inputs):
    out, _ = run_kernel(inputs)
    return out


def run_kernel(inputs, **spmd_kwargs):
    global _NC
    if _NC is None:
        _NC = build_nc()

    query = np.asarray(inputs["query"], np.float32)
    key = np.asarray(inputs["key"], np.float32)
    value = np.asarray(inputs["value"], np.float32)
    wv = np.asarray(inputs["wv"], np.float32)
    bv = np.asarray(inputs["bv"], np.float32)
    M, c = host_consts(
        np.asarray(inputs["wq"], np.float32), np.asarray(inputs["bq"], np.float32),
        np.asarray(inputs["wk"], np.float32), np.asarray(inputs["bk"], np.float32),
        wv, bv)

    B = query.shape[0]
    in_maps = []
    for b in range(B):
        xqT, ktil, vaug = host_pack(query[b], key[b], value[b], M, c)
        in_maps.append({"xqT": xqT, "ktil": ktil, "vaug": vaug})
    res = run_bass_kernel_spmd(_NC, in_maps, core_ids=list(range(B)), **spmd_kwargs)
    outs = []
    for b in range(B):
        u2 = res.results[b]["uout"]             # [65, 2L]: per-chunk half-sums
        u2 = u2.reshape(F + 1, NCH, 2, CHW)
        u = (u2[:, :, 0, :] + u2[:, :, 1, :]).reshape(F + 1, L)
        ut = (u[0:F, :] / u[F:F + 1, :]).T      # [L, F] normalized attn @ value
        outs.append(ut @ wv.T + bv)             # host fp32 epilogue projection
    out = np.stack(outs).astype(np.float32)
    return out, res


# revision 14
# speedup vs baseline: 1.6003x; 1.0303x over previous
"""Trainium2 Bass kernel for nn_DotProductAttention (B=8, LQ=LK=4096, F=64).

Reference computation:
    q = query @ wq.T + bq ; k = key @ wk.T + bk ; v = value @ wv.T + bv
    scores = einsum('bkf,bqf->bkq', k, q)
    attn = softmax(scores, axis=-1)           # over q positions
    out = einsum('bkq,bqf->bkf', attn, v)

Strategy: batch b -> core b (8 cores, no cross-core communication).

Algebraic folding (host side, O(L*F) prep only -- all O(L^2) work on device):
    with M = wq^T wk, c = wq^T bk (per-k softmax-invariant term dropped):
        S^T[q,k] = query[q,:] @ ktil[:,k],   ktil = M @ key^T + c   (host)
    U^T = [value | 1]^T @ exp(S^T) accumulates in PSUM; its last row is the
    softmax denominator l. The tiny projection (U/l) @ wv.T + bv runs on host.

Device loop (per core): 8 k-chunks of 512 columns; per chunk sweep the 32
q-blocks (slots). The exp() of the 16.7M scores is the kernel bottleneck
(ACT engine: 1 elem/cycle/lane @1.2GHz = 109us alone), so exp is SPLIT:
  - ACT path: exact Exp ACTIVATEs over the leading banks of each PSUM
    supertile -> bf16 SBUF.
  - DVE path (~10 of 32 slots/chunk): the third bank of an "M" supertile is
    processed by one tensor_scalar: i16 = round(s * 128*log2(e) + B) from
    PSUM fp32 [128,512]; the int16 buffer IS the bf16 bit pattern of ~e^s
    (Schraudolph bit-trick exp2, ~3.4% max elem err; ~1.2e-2 output err at
    ~30% routing -- inside the 2e-2 budget). ScalarE/VectorE read different
    PSUM banks of the same tile in parallel (legal on TRN2).
P@V lags LAG slots behind scores: two concurrent 64-row-group matmuls
(tile_position row tiling) accumulate q-halves into utl/uth (separate PSUM
banks -- concurrent drains into one bank are fatal). Host adds the halves.
PSUM: 2x3 supertile banks + utl + uth = 8.
"""

import numpy as np
import ml_dtypes

import concourse.mybir as mybir
import concourse.tile as tile
from concourse import bacc
from concourse.bass_utils import run_bass_kernel_spmd
from concourse.vector_clock import ScopedClock


class _FastExitTileContext(tile.TileContext):
    """TileContext whose exit skips the second all-engine barrier.

    The final barrier only orders the gpsimd semaphore-clears against the
    other engines' completion; NEFF execution completion already waits for
    every engine's last instruction, and the clears still run, so repeated
    executions stay correct. Saves ~2-3us of kernel tail.
    """

    def _drain_and_barrier(self, tick_clock, wait_clock):
        drain_inst = self.nc.sync.drain()
        wait_clock.add_sem_waits(
            drain_inst.ins, ScopedClock({None: tick_clock.global_clock})
        )
        self.nc.all_engine_barrier()
        popped = self.nc._tile_sem_poison_stack.pop()
        assert popped is self._sem_poison
        self.nc.clear_and_free_semaphores(list(self.sems.allocated().values()))


F32 = mybir.dt.float32
F16 = mybir.dt.float16
BF16 = mybir.dt.bfloat16
I16 = mybir.dt.int16

L = 4096          # sequence length (both q and k)
F = 64            # feature dim
NJ = 32           # q-blocks of 128 (slots per chunk)
CHW = 512         # k-chunk width
NCH = 8           # number of k-chunks

# DVE Schraudolph constants: i16 = round(s * SCHRAU_SCALE + SCHRAU_B)
SCHRAU_SCALE = float(128.0 * np.log2(np.e))   # 184.664...
SCHRAU_B = float(16256 - 6)                   # 127<<7 minus sawtooth centering

LAG = 8           # P@V emission lag in slots


def chunk_layout(c):
    """Per-chunk supertile list: each entry is a list of per-slot kinds.

    'A' slots feed one exact-exp ACTIVATE (contiguous leading cols of the
    tile); a trailing 'D' slot is the tile's last bank, consumed by the DVE
    Schraudolph op. Steady chunks: 9x[A,A,D] + [A,A,A] + [A,D] -> 10 D slots.
    Chunk 0 leads with small ACT groups to prime the pipeline; the last
    chunk mirrors so the tail drains fast.
    """
    m, ad = ["D", "A", "A"], ["D", "A"]
    if c == 0:
        pat = [["A"], ["A"], ["A", "A"], ["A", "A"]] + [m] * 8 + [ad]
    elif c == NCH - 1:
        pat = [m] * 9 + [ad, ["A", "A"], ["A"]]
    else:
        pat = [m] * 10 + [["A", "A"]]
    assert sum(len(t) for t in pat) == NJ
    return pat


def build_nc():
    nc = bacc.Bacc(None, target_bir_lowering=False)

    xqT = nc.dram_tensor("xqT", [128, L // 2], F16, kind="ExternalInput")
    ktil = nc.dram_tensor("ktil", [128, L], F16, kind="ExternalInput")
    vaug = nc.dram_tensor("vaug", [128, NJ * (F + 1)], BF16, kind="ExternalInput")
    # Per chunk c: cols [1024c, 1024c+512) = low-q-half partial sums,
    # [1024c+512, 1024c+1024) = high-q-half; host adds the halves.
    uout = nc.dram_tensor("uout", [F + 1, 2 * L], F32, kind="ExternalOutput")

    Exp = mybir.ActivationFunctionType.Exp
    Mult = mybir.AluOpType.mult
    Add = mybir.AluOpType.add

    # slot -> (kind, tile_key, offset_in_tile, tile_kinds, a_start)
    layouts = {}
    for c in range(NCH):
        smap = {}
        s = 0
        for gi, kinds in enumerate(chunk_layout(c)):
            for off, kind in enumerate(kinds):
                smap[s] = (kind, (c, gi), off, kinds, kinds.index("A"))
                s += 1
        layouts[c] = smap

    with _FastExitTileContext(nc) as tc:
        with (
            tc.tile_pool(name="persist", bufs=1) as persist,
            tc.tile_pool(name="pt", bufs=6) as ptpool,
            tc.tile_pool(name="dpt", bufs=6) as dptpool,
            tc.tile_pool(name="utbf", bufs=2) as utbfpool,
            tc.tile_pool(name="ps_st", bufs=2, space="PSUM") as ps_st,
            tc.tile_pool(name="ps_ut", bufs=2, space="PSUM") as ps_ut,
        ):
            # Warm the ACT exp table while input DMAs are still in flight.
            warm_in = persist.tile([128, 8], F32)
            warm_out = persist.tile([128, 8], BF16)
            nc.vector.memset(warm_in[:], 0.0)
            nc.scalar.activation(warm_out[:], warm_in[:], Exp)

            # Split DMAs so the first iterations' inputs land early.
            xqT_sb = persist.tile([128, L // 2], F16)
            ktil_sb = persist.tile([128, L], F16)
            vaug_sb = persist.tile([128, NJ * (F + 1)], BF16)
            nc.sync.dma_start(xqT_sb[0:64, 0:128], xqT[0:64, 0:128])
            nc.sync.dma_start(ktil_sb[0:64, 0:CHW], ktil[0:64, 0:CHW])
            nc.sync.dma_start(xqT_sb[64:128, 0:128], xqT[64:128, 0:128])
            nc.sync.dma_start(ktil_sb[64:128, 0:CHW], ktil[64:128, 0:CHW])
            nc.sync.dma_start(vaug_sb[:, 0:2 * (F + 1)], vaug[:, 0:2 * (F + 1)])
            nc.sync.dma_start(xqT_sb[:, 128:], xqT[:, 128:])
            nc.sync.dma_start(vaug_sb[:, 2 * (F + 1):], vaug[:, 2 * (F + 1):])
            nc.sync.dma_start(ktil_sb[:, CHW:], ktil[:, CHW:])

            sts = {}    # tile_key -> psum supertile
            pta = {}    # tile_key -> ACT bf16 exp tile
            ptd = {}    # tile_key -> DVE i16 exp tile
            uts = {}    # chunk -> (utl, uth) psum accumulators

            def emit_scores(c, s):
                kind, key, off, kinds, a0 = layouts[c][s]
                n_a = len(kinds) - a0
                if off == 0:
                    # The D slot gets its OWN tile (separate tag): ScalarE and
                    # VectorE then never read the same tile, avoiding Tile's
                    # conservative cross-engine read ordering (ACT stalling on
                    # the DVE TS). Banks/buffer budget unchanged: 2x(1+2).
                    std = (ps_st.tile([128, CHW], F32, name="std", tag="stD")
                           if a0 > 0 else None)
                    sta = ps_st.tile([128, CHW * n_a], F32, name="sta", tag="stA")
                    sts[key] = (std, sta)
                std, sta = sts[key]
                rh = 64 * (s % 2)
                qcols = slice(128 * (s // 2), 128 * (s // 2 + 1))
                kcols = slice(CHW * c, CHW * (c + 1))
                dst = (std[:, :] if kind == "D"
                       else sta[:, CHW * (off - a0): CHW * (off - a0 + 1)])
                nc.tensor.matmul(dst,
                                 xqT_sb[rh:rh + 64, qcols],
                                 ktil_sb[rh:rh + 64, kcols],
                                 start=True, stop=True, tile_position=(rh, 0))
                if kind == "D":
                    dpt = dptpool.tile([128, CHW], I16, name="dpt", tag="dpt")
                    nc.vector.tensor_scalar(
                        dpt[:], std[:, :],
                        SCHRAU_SCALE, SCHRAU_B, Mult, Add)
                    ptd[key] = dpt
                if off == len(kinds) - 1:
                    pt = ptpool.tile([128, CHW * n_a], BF16, name="pt", tag="pt")
                    nc.scalar.activation(pt[:], sta[:, :], Exp)
                    pta[key] = pt
                    sts.pop(key)

            def emit_pav(c, s):
                if s == 0:
                    uts[c] = (ps_ut.tile([F + 1, CHW], F32, name="utl", tag="ut"),
                              ps_ut.tile([F + 1, CHW], F32, name="uth", tag="ut"))
                utl, uth = uts[c]
                kind, key, off, kinds, a0 = layouts[c][s]
                if kind == "A":
                    rhs = pta[key][:, CHW * (off - a0): CHW * (off - a0 + 1)]
                else:
                    rhs = ptd[key][:].bitcast(BF16)
                vsl = slice((F + 1) * s, (F + 1) * (s + 1))
                # Emit the half OPPOSITE to this slot's scores row-group
                # first: the PE stream then alternates h0/h64 on every
                # adjacent matmul, so row-tiled pairing is continuous.
                halves = [
                    (utl, vaug_sb[0:64, vsl], rhs[0:64, :], (0, 0)),
                    (uth, vaug_sb[64:128, vsl], rhs[64:128, :], (64, 0)),
                ]
                for out, w, r, tp in (halves if s % 2 else halves[::-1]):
                    nc.tensor.matmul(out[:], w, r,
                                     start=(s == 0), stop=(s == NJ - 1),
                                     tile_position=tp)
                if kind == "A" and off == len(kinds) - 1:
                    pta.pop(key)
                if kind == "D":
                    ptd.pop(key)

            def emit_epilogue(c):
                utl, uth = uts.pop(c)
                utbf = utbfpool.tile([F + 1, 2 * CHW], F32)
                # copy+DMA per half so the second copy overlaps the first DMA
                nc.vector.tensor_copy(utbf[:, 0:CHW], utl[:])
                nc.sync.dma_start(uout[:, 2 * CHW * c: 2 * CHW * c + CHW],
                                  utbf[:, 0:CHW])
                nc.vector.tensor_copy(utbf[:, CHW:], uth[:])
                nc.sync.dma_start(uout[:, 2 * CHW * c + CHW: 2 * CHW * (c + 1)],
                                  utbf[:, CHW:])

            # P@V lags scores by LAG slots; each chunk's first BOOST_N pavs
            # get BOOST extra lag (with later catch-up) so the accumulator-
            # bank WAR stall at chunk transitions can't head-of-line-block
            # the scores feeding the next ACTIVATEs.
            BOOST, BOOST_N = 4, 4
            NTOT = NCH * NJ
            emitted = 0
            for gstep in range(NTOT + LAG + BOOST + 1):
                if gstep < NTOT:
                    emit_scores(gstep // NJ, gstep % NJ)
                budget = 2
                while emitted < NTOT and budget > 0:
                    pc, ps = emitted // NJ, emitted % NJ
                    need = emitted + LAG + (BOOST if ps < BOOST_N else 0)
                    if gstep < need:
                        break
                    emit_pav(pc, ps)
                    if ps == NJ - 1:
                        emit_epilogue(pc)
                    emitted += 1
                    budget -= 1
            assert emitted == NTOT

    nc.compile()
    return nc


def host_pack(query_b, key_b, value_b, M, c):
    """Per-batch device-input packing (numpy, O(L*F))."""
    qT = query_b.T.reshape(F, L // 256, 2, 128)
    xqT = np.ascontiguousarray(                                       # [128, L/2]
        np.concatenate([qT[:, :, 0, :], qT[:, :, 1, :]], axis=0)
        .reshape(128, L // 2)).astype(np.float16)
    kt = (M @ key_b.T + c[:, None]).astype(np.float16)                # [64, L]
    ktil = np.ascontiguousarray(np.concatenate([kt, kt], axis=0))     # [128, L]
    v3 = value_b.reshape(NJ, 128, F).transpose(1, 0, 2)               # [128, NJ, F]
    vaug = np.ones((128, NJ, F + 1), np.float32)
    vaug[:, :, 0:F] = v3
    vaug_bf = vaug.reshape(128, NJ * (F + 1)).astype(ml_dtypes.bfloat16)
    return xqT, ktil, np.ascontiguousarray(vaug_bf)


def host_consts(wq, bq, wk, bk, wv, bv):
    wq64 = wq.astype(np.float64)
    M = (wq64.T @ wk.astype(np.float64)).astype(np.float32)
    c = (wq64.T @ bk.astype(np.float64)).astype(np.float32)
    return M, c


_NC = None


def kernel(**inputs):
    out, _ = run_kernel(inputs)
    return out


def run_kernel(inputs, **spmd_kwargs):
    global _NC
    if _NC is None:
        _NC = build_nc()

    query = np.asarray(inputs["query"], np.float32)
    key = np.asarray(inputs["key"], np.float32)
    value = np.asarray(inputs["value"], np.float32)
    wv = np.asarray(inputs["wv"], np.float32)
    bv = np.asarray(inputs["bv"], np.float32)
    M, c = host_consts(
        np.asarray(inputs["wq"], np.float32), np.asarray(inputs["bq"], np.float32),
        np.asarray(inputs["wk"], np.float32), np.asarray(inputs["bk"], np.float32),
        wv, bv)

    B = query.shape[0]
    in_maps = []
    for b in range(B):
        xqT, ktil, vaug = host_pack(query[b], key[b], value[b], M, c)
        in_maps.append({"xqT": xqT, "ktil": ktil, "vaug": vaug})
    res = run_bass_kernel_spmd(_NC, in_maps, core_ids=list(range(B)), **spmd_kwargs)
    outs = []
    for b in range(B):
        u2 = res.results[b]["uout"]             # [65, 2L]: per-chunk half-sums
        u2 = u2.reshape(F + 1, NCH, 2, CHW)
        u = (u2[:, :, 0, :] + u2[:, :, 1, :]).reshape(F + 1, L)
        ut = (u[0:F, :] / u[F:F + 1, :]).T      # [L, F] normalized attn @ value
        outs.append(ut @ wv.T + bv)             # host fp32 epilogue projection
    out = np.stack(outs).astype(np.float32)
    return out, res
